# revision 8
# baseline (speedup 1.0000x reference)
"""Causal self-attention (B=4, T=2048, C=1024, H=16) on 8 TRN2 NeuronCores.

Sharding: core = (batch, head-group) on a 4x2 grid. Each core computes the
attention output of 8 heads for one batch element plus its partial out-proj
(y^T = w_out_slice^T @ out_heads^T); the two head-groups of a batch are summed
on the host (the "out_proj all-reduce"), where the final bias is also added.

On-chip dataflow is fully transposed so no transposes are ever needed:
  qk^T  = w_qkv_slice^T @ x^T          (C on partitions)
  v     = x @ w_v_slice                (T on partitions, natural)
  S^T   = k_h @ q_h^T                  (k-positions on partitions)
  P^T   = exp(S^T) * causal_mask       (no max-subtraction: scores ~ N(0,1))
  outT  = [v|1]^T @ P^T                (ones column accumulates sum-of-exp)
  y^T   = w_out_slice^T @ (outT/sumexp)
"""

import sys
import types

if "/opt/trn_rl_repo" not in sys.path:
    sys.path.insert(0, "/opt/trn_rl_repo")

import numpy as np


def _install_ntff_hook_shim():
    """antenv.axon_hooks is missing in this image; provide it so that
    run_bass_kernel_spmd(trace=True) can capture NTFF profiles."""
    if "antenv.axon_hooks" in sys.modules:
        return
    try:
        from trn_agent_boot.trn_boot import _ntff_profile_via_ctypes

        hook = _ntff_profile_via_ctypes("/opt/axon/libaxon_pjrt.so")
    except Exception:
        hook = None
    m = types.ModuleType("antenv.axon_hooks")
    m.get_axon_ntff_profile_hook = lambda: hook
    sys.modules["antenv.axon_hooks"] = m


_install_ntff_hook_shim()

import concourse.bass as bass  # noqa: E402
from concourse import bacc  # noqa: E402
import concourse.mybir as mybir  # noqa: E402
import concourse.tile as tile  # noqa: E402
from concourse.bass_utils import run_bass_kernel_spmd  # noqa: E402

BF16 = mybir.dt.bfloat16
F32 = mybir.dt.float32
NPBF16 = mybir.dt.np(BF16)
EXP = mybir.ActivationFunctionType.Exp

B, T, C = 4, 2048, 1024
H, DH = 16, 64
HC = 8           # heads per core
CK = C // 128    # 8 contraction chunks over C
TB = T // 128    # 16 key blocks / T row blocks
QC = T // 512    # 4 query chunks
SCALE = 1.0 / np.sqrt(DH)

TRACE = False          # set True (e.g. from test.py) to capture an NTFF profile
LAST_RESULT = None     # BassKernelResults of the last run (exec_time_ns etc.)

_CACHE = None


def _build():
    """Build + compile the single-core Bass program (SPMD across 8 cores)."""
    nc = bacc.Bacc("TRN2", target_bir_lowering=False, debug=False, num_devices=8)

    xT = nc.dram_tensor("xT", [C, T], BF16, kind="ExternalInput")
    wqkv = nc.dram_tensor("wqkv", [C, 3 * 512], BF16, kind="ExternalInput")
    bqk = nc.dram_tensor("bqk", [128, CK], F32, kind="ExternalInput")
    bv = nc.dram_tensor("bv", [64, HC], F32, kind="ExternalInput")
    wout = nc.dram_tensor("wout", [512, C], BF16, kind="ExternalInput")
    msk = nc.dram_tensor("msk", [128, 4, 512], BF16, kind="ExternalInput")
    yT = nc.dram_tensor("yT", [C, T], F32, kind="ExternalOutput")

    with tile.TileContext(nc) as tc:
        with tc.tile_pool(name="persist", bufs=1) as pp:
            # qk^T: chunks 0-3 = q cols (head h at partition-offset (h%2)*64 of
            # chunk h//2), chunks 4-7 = k cols likewise.
            QKT = pp.tile([128, CK, T], BF16, tag="qkt")
            # v blocks, per T-block: 8 heads x (64 v-dims + ones column)
            VA = pp.tile([128, TB, HC * 65], BF16, tag="va")
            MASK = pp.tile([128, 4, 512], BF16, tag="mask")
            # out_heads^T: head h at [(h%2)*64 :, h//2, :]
            OT = pp.tile([128, 4, T], BF16, tag="ot")
            WOUT = pp.tile([128, 4, C], BF16, tag="wout")
            BQK = pp.tile([128, CK], F32, tag="bqk")
            BV = pp.tile([64, HC], F32, tag="bv")

            nc.sync.dma_start(MASK[:], msk[:])
            nc.sync.dma_start(BQK[:], bqk[:])
            nc.sync.dma_start(BV[:], bv[:])
            for kc in range(4):
                nc.sync.dma_start(WOUT[:, kc, :], wout[kc * 128:(kc + 1) * 128, :])
            # ones columns of VA; v copies below overwrite the rest
            nc.vector.memset(VA[:], 1.0)

            # ---------------- Stage A: qkv projection ----------------
            with (
                tc.tile_pool(name="sa", bufs=1) as sa,
                tc.tile_pool(name="psA", bufs=2, space="PSUM") as psA,
            ):
                XT = sa.tile([128, CK, T], BF16, tag="xt")
                WQ = sa.tile([128, CK, 1536], BF16, tag="wq")
                for kc in range(CK):
                    nc.sync.dma_start(XT[:, kc, :], xT[kc * 128:(kc + 1) * 128, :])
                    nc.sync.dma_start(WQ[:, kc, :], wqkv[kc * 128:(kc + 1) * 128, :])

                def emit_qk(psl, m, n):
                    for kc in range(CK):
                        nc.tensor.matmul(
                            psl,
                            WQ[:, kc, m * 128:(m + 1) * 128],
                            XT[:, kc, n * 512:(n + 1) * 512],
                            start=(kc == 0),
                            stop=(kc == CK - 1),
                        )
                    nc.vector.tensor_scalar_add(
                        QKT[:, m, n * 512:(n + 1) * 512], psl, BQK[:, m:m + 1]
                    )

                def emit_v(psl, t):
                    for kc in range(CK):
                        nc.tensor.matmul(
                            psl,
                            XT[:, kc, t * 128:(t + 1) * 128],
                            WQ[:, kc, 1024:1536],
                            start=(kc == 0),
                            stop=(kc == CK - 1),
                        )
                    src = psl.rearrange("p (h c) -> p h c", c=64)
                    dst = VA[:, t, :].rearrange("p (h c) -> p h c", c=65)[:, :, 0:64]
                    nc.vector.tensor_copy(dst, src)

                jobs = [("qk", m, n) for m in range(8) for n in range(4)]
                jobs += [("v", t, 0) for t in range(TB)]
                for g0 in range(0, len(jobs), 3):
                    t3 = psA.tile([128, 1536], F32, tag="ps")
                    for s, (kind, a, b_) in enumerate(jobs[g0:g0 + 3]):
                        psl = t3[:, s * 512:(s + 1) * 512]
                        if kind == "qk":
                            emit_qk(psl, a, b_)
                        else:
                            emit_v(psl, a)

            # ---------------- Stage B: attention ----------------
            with (
                tc.tile_pool(name="sc", bufs=2, space="PSUM") as scp,
                tc.tile_pool(name="oa", bufs=2, space="PSUM") as oap,
                tc.tile_pool(name="pt", bufs=3) as ptp,
                tc.tile_pool(name="nrm", bufs=2) as nrm,
            ):
                for pair in range(4):
                    heads = (2 * pair, 2 * pair + 1)
                    for j in range(QC):
                        nb = 4 * (j + 1)  # causal: key blocks 0..nb-1
                        oaccs = {h: oap.tile([65, 512], F32, tag="oacc", name=f"oacc{h}") for h in heads}
                        for g0 in range(0, nb, 3):
                            gn = min(3, nb - g0)
                            scs = {h: scp.tile([128, 1536], F32, tag="sc", name=f"sc{h}") for h in heads}
                            # scores: interleave the two heads so their K=64
                            # matmuls land on different PE row groups and run
                            # concurrently.
                            for b2 in range(gn):
                                i = g0 + b2
                                for h in heads:
                                    po = (h % 2) * 64
                                    mc = h // 2
                                    nc.tensor.matmul(
                                        scs[h][:, b2 * 512:(b2 + 1) * 512],
                                        QKT[po:po + 64, 4 + mc, i * 128:(i + 1) * 128],
                                        QKT[po:po + 64, mc, j * 512:(j + 1) * 512],
                                        start=True,
                                        stop=True,
                                    )
                            pts = {}
                            for h in heads:
                                pt = ptp.tile([128, 1536], BF16, tag="pt")
                                nc.scalar.activation(
                                    pt[:, 0:gn * 512], scs[h][:, 0:gn * 512], EXP
                                )
                                for b2 in range(gn):
                                    d = g0 + b2 - 4 * j
                                    if d >= 0:
                                        sl = pt[:, b2 * 512:(b2 + 1) * 512]
                                        nc.vector.tensor_mul(sl, sl, MASK[:, d, :])
                                pts[h] = pt
                            for b2 in range(gn):
                                i = g0 + b2
                                for h in heads:
                                    nc.tensor.matmul(
                                        oaccs[h],
                                        VA[:, i, h * 65:h * 65 + 65],
                                        pts[h][:, b2 * 512:(b2 + 1) * 512],
                                        start=(i == 0),
                                        stop=(i == nb - 1),
                                    )
                        for h in heads:
                            po = (h % 2) * 64
                            mc = h // 2
                            rc = nrm.tile([65, 512], F32, tag="rc")
                            nc.vector.reciprocal(rc[64:65, :], oaccs[h][64:65, :])
                            rb = nrm.tile([64, 512], F32, tag="rb")
                            row = rc[64:65, :]
                            row_bcast = bass.AP(
                                tensor=row.tensor,
                                offset=row.offset,
                                ap=[row.ap[0], [0, 64]] + row.ap[1:],
                            )
                            nc.gpsimd.dma_start(out=rb[:], in_=row_bcast)
                            ost = nrm.tile([64, 512], BF16, tag="ost")
                            nc.vector.tensor_mul(ost[:], oaccs[h][0:64, :], rb[:])
                            nc.vector.tensor_scalar_add(ost[:], ost[:], BV[:, h:h + 1])
                            nc.sync.dma_start(
                                OT[po:po + 64, mc, j * 512:(j + 1) * 512], ost[:]
                            )

            # ---------------- Stage C: output projection ----------------
            with (
                tc.tile_pool(name="psC", bufs=2, space="PSUM") as psC,
                tc.tile_pool(name="yst", bufs=3) as yst,
            ):
                jobs = [(mo, n) for mo in range(8) for n in range(4)]
                for g0 in range(0, len(jobs), 3):
                    t3 = psC.tile([128, 1536], F32, tag="psc")
                    for s, (mo, n) in enumerate(jobs[g0:g0 + 3]):
                        psl = t3[:, s * 512:(s + 1) * 512]
                        for kc in range(4):
                            nc.tensor.matmul(
                                psl,
                                WOUT[:, kc, mo * 128:(mo + 1) * 128],
                                OT[:, kc, n * 512:(n + 1) * 512],
                                start=(kc == 0),
                                stop=(kc == 3),
                            )
                        ys = yst.tile([128, 512], F32, tag="ys")
                        nc.vector.tensor_copy(ys[:], psl)
                        nc.sync.dma_start(
                            yT[mo * 128:(mo + 1) * 128, n * 512:(n + 1) * 512], ys[:]
                        )

    nc.compile()
    return nc


def _make_masks():
    p = np.arange(128)[:, None, None]
    d = np.arange(4)[None, :, None] * 128
    f = np.arange(512)[None, None, :]
    return (d + p <= f).astype(np.float32).astype(NPBF16)


def kernel(x, w_qkv, b_qkv, w_out, b_out):
    global _CACHE, LAST_RESULT
    x = np.asarray(x, np.float32)
    w_qkv = np.asarray(w_qkv, np.float32)
    b_qkv = np.asarray(b_qkv, np.float32)
    w_out = np.asarray(w_out, np.float32)
    b_out = np.asarray(b_out, np.float32)

    if _CACHE is None:
        _CACHE = _build()
    nc = _CACHE

    masks = _make_masks()
    in_maps = []
    for core in range(8):
        b = core // 2
        g = core % 2
        sl = slice(g * 512, (g + 1) * 512)
        wq = w_qkv[:, 0:1024][:, sl] * SCALE
        wk = w_qkv[:, 1024:2048][:, sl]
        wv = w_qkv[:, 2048:3072][:, sl]
        wqkv_c = np.ascontiguousarray(
            np.concatenate([wq, wk, wv], axis=1).astype(NPBF16)
        )
        bq = b_qkv[0:1024][sl] * SCALE
        bk = b_qkv[1024:2048][sl]
        bqk_c = np.ascontiguousarray(
            np.concatenate([bq, bk]).reshape(CK, 128).T.astype(np.float32)
        )
        bv_c = np.ascontiguousarray(
            b_qkv[2048:3072][sl].reshape(HC, 64).T.astype(np.float32)
        )
        in_maps.append(
            {
                "xT": np.ascontiguousarray(x[b].T.astype(NPBF16)),
                "wqkv": wqkv_c,
                "bqk": bqk_c,
                "bv": bv_c,
                "wout": np.ascontiguousarray(w_out[sl, :].astype(NPBF16)),
                "msk": masks,
            }
        )

    res = run_bass_kernel_spmd(nc, in_maps, core_ids=list(range(8)), trace=TRACE)
    LAST_RESULT = res

    out = np.empty((B, T, C), np.float32)
    for b in range(B):
        acc = res.results[2 * b]["yT"].astype(np.float32) + res.results[
            2 * b + 1
        ]["yT"].astype(np.float32)
        out[b] = acc.T + b_out[None, :]
    return out


# revision 11
# speedup vs baseline: 1.0973x; 1.0973x over previous
"""Causal self-attention (B=4, T=2048, C=1024, H=16) on 8 TRN2 NeuronCores.

Sharding: core = (batch, head-group) on a 4x2 grid. Each core computes the
attention output of 8 heads for one batch element plus its partial out-proj
(y^T = w_out_slice^T @ out_heads^T); the two head-groups of a batch are summed
on the host (the "out_proj all-reduce"), where the final bias is also added.

On-chip dataflow is fully transposed so no transposes are ever needed:
  qk^T  = w_qkv_slice^T @ x^T          (C on partitions)
  v     = x @ w_v_slice                (T on partitions, natural)
  S^T   = k_h @ q_h^T                  (k-positions on partitions)
  P^T   = exp(S^T) * causal_mask       (no max-subtraction: scores ~ N(0,1))
  outT  = [v|1]^T @ P^T                (ones column accumulates sum-of-exp)
  y^T   = w_out_slice^T @ (outT/sumexp)
"""

import sys
import types

if "/opt/trn_rl_repo" not in sys.path:
    sys.path.insert(0, "/opt/trn_rl_repo")

import numpy as np


def _install_ntff_hook_shim():
    """antenv.axon_hooks is missing in this image; provide it so that
    run_bass_kernel_spmd(trace=True) can capture NTFF profiles."""
    if "antenv.axon_hooks" in sys.modules:
        return
    try:
        from trn_agent_boot.trn_boot import _ntff_profile_via_ctypes

        hook = _ntff_profile_via_ctypes("/opt/axon/libaxon_pjrt.so")
    except Exception:
        hook = None
    m = types.ModuleType("antenv.axon_hooks")
    m.get_axon_ntff_profile_hook = lambda: hook
    sys.modules["antenv.axon_hooks"] = m


_install_ntff_hook_shim()

import concourse.bass as bass  # noqa: E402
from concourse import bacc  # noqa: E402
import concourse.mybir as mybir  # noqa: E402
import concourse.tile as tile  # noqa: E402
from concourse.bass_utils import run_bass_kernel_spmd  # noqa: E402

BF16 = mybir.dt.bfloat16
F32 = mybir.dt.float32
NPBF16 = mybir.dt.np(BF16)
EXP = mybir.ActivationFunctionType.Exp

B, T, C = 4, 2048, 1024
H, DH = 16, 64
HC = 8           # heads per core
CK = C // 128    # 8 contraction chunks over C
TB = T // 128    # 16 key blocks / T row blocks
QC = T // 512    # 4 query chunks
SCALE = 1.0 / np.sqrt(DH)

TRACE = False          # set True (e.g. from test.py) to capture an NTFF profile
LAST_RESULT = None     # BassKernelResults of the last run (exec_time_ns etc.)

_CACHE = None


def _build():
    """Build + compile the single-core Bass program (SPMD across 8 cores)."""
    nc = bacc.Bacc("TRN2", target_bir_lowering=False, debug=False, num_devices=8)

    xT = nc.dram_tensor("xT", [C, T], BF16, kind="ExternalInput")
    wqkv = nc.dram_tensor("wqkv", [C, 3 * 512], BF16, kind="ExternalInput")
    bqk = nc.dram_tensor("bqk", [128, CK], F32, kind="ExternalInput")
    bv = nc.dram_tensor("bv", [64, HC], F32, kind="ExternalInput")
    wout = nc.dram_tensor("wout", [512, C], BF16, kind="ExternalInput")
    msk = nc.dram_tensor("msk", [128, 4, 512], BF16, kind="ExternalInput")
    yT = nc.dram_tensor("yT", [C, T], F32, kind="ExternalOutput")

    with tile.TileContext(nc) as tc:
        with tc.tile_pool(name="persist", bufs=1) as pp:
            # qk^T: chunks 0-3 = q cols (head h at partition-offset (h%2)*64 of
            # chunk h//2), chunks 4-7 = k cols likewise.
            QKT = pp.tile([128, CK, T], BF16, tag="qkt")
            # v blocks, per T-block: 8 heads x (64 v-dims + ones column)
            VA = pp.tile([128, TB, HC * 65], BF16, tag="va")
            MASK = pp.tile([128, 4, 512], BF16, tag="mask")
            # out_heads^T: head h at [(h%2)*64 :, h//2, :]
            OT = pp.tile([128, 4, T], BF16, tag="ot")
            WOUT = pp.tile([128, 4, C], BF16, tag="wout")
            BQK = pp.tile([128, CK], F32, tag="bqk")
            BV = pp.tile([64, HC], F32, tag="bv")

            nc.sync.dma_start(MASK[:], msk[:])
            nc.sync.dma_start(BQK[:], bqk[:])
            nc.sync.dma_start(BV[:], bv[:])
            for kc in range(4):
                nc.sync.dma_start(WOUT[:, kc, :], wout[kc * 128:(kc + 1) * 128, :])
            # ones columns of VA; v copies below overwrite the rest
            nc.vector.memset(VA[:], 1.0)

            # ---------------- Stage A: qkv projection ----------------
            with (
                tc.tile_pool(name="sa", bufs=1) as sa,
                tc.tile_pool(name="psA", bufs=2, space="PSUM") as psA,
            ):
                XT = sa.tile([128, CK, T], BF16, tag="xt")
                WQ = sa.tile([128, CK, 1536], BF16, tag="wq")
                for kc in range(CK):
                    nc.sync.dma_start(XT[:, kc, :], xT[kc * 128:(kc + 1) * 128, :])
                    nc.sync.dma_start(WQ[:, kc, :], wqkv[kc * 128:(kc + 1) * 128, :])

                def emit_qk(psl, m, n):
                    for kc in range(CK):
                        nc.tensor.matmul(
                            psl,
                            WQ[:, kc, m * 128:(m + 1) * 128],
                            XT[:, kc, n * 512:(n + 1) * 512],
                            start=(kc == 0),
                            stop=(kc == CK - 1),
                        )
                    nc.vector.tensor_scalar_add(
                        QKT[:, m, n * 512:(n + 1) * 512], psl, BQK[:, m:m + 1]
                    )

                def emit_v(psl, t):
                    for kc in range(CK):
                        nc.tensor.matmul(
                            psl,
                            XT[:, kc, t * 128:(t + 1) * 128],
                            WQ[:, kc, 1024:1536],
                            start=(kc == 0),
                            stop=(kc == CK - 1),
                        )
                    src = psl.rearrange("p (h c) -> p h c", c=64)
                    dst = VA[:, t, :].rearrange("p (h c) -> p h c", c=65)[:, :, 0:64]
                    nc.vector.tensor_copy(dst, src)

                jobs = [("qk", m, n) for m in range(8) for n in range(4)]
                jobs += [("v", t, 0) for t in range(TB)]
                for g0 in range(0, len(jobs), 3):
                    t3 = psA.tile([128, 1536], F32, tag="ps")
                    for s, (kind, a, b_) in enumerate(jobs[g0:g0 + 3]):
                        psl = t3[:, s * 512:(s + 1) * 512]
                        if kind == "qk":
                            emit_qk(psl, a, b_)
                        else:
                            emit_v(psl, a)

            # ---------------- Stage B: attention ----------------
            with (
                tc.tile_pool(name="sc", bufs=2, space="PSUM") as scp,
                tc.tile_pool(name="oa", bufs=4, space="PSUM") as oap,
                tc.tile_pool(name="pt", bufs=3) as ptp,
                tc.tile_pool(name="nrm", bufs=3) as nrm,
            ):
                for pair in range(4):
                    heads = (2 * pair, 2 * pair + 1)
                    for j in range(QC):
                        nb = 4 * (j + 1)  # causal: key blocks 0..nb-1
                        oaccs = {h: oap.tile([65, 512], F32, tag="oacc", name=f"oacc{h}") for h in heads}
                        for g0 in range(0, nb, 2):
                            gn = min(2, nb - g0)
                            scs = {h: scp.tile([128, 1024], F32, tag="sc", name=f"sc{h}") for h in heads}
                            # scores: interleave the two heads so their K=64
                            # matmuls land on different PE row groups and run
                            # concurrently.
                            for b2 in range(gn):
                                i = g0 + b2
                                for h in heads:
                                    po = (h % 2) * 64
                                    mc = h // 2
                                    nc.tensor.matmul(
                                        scs[h][:, b2 * 512:(b2 + 1) * 512],
                                        QKT[po:po + 64, 4 + mc, i * 128:(i + 1) * 128],
                                        QKT[po:po + 64, mc, j * 512:(j + 1) * 512],
                                        start=True,
                                        stop=True,
                                    )
                            pts = {}
                            for h in heads:
                                pt = ptp.tile([128, 1024], BF16, tag="pt")
                                nc.scalar.activation(
                                    pt[:, 0:gn * 512], scs[h][:, 0:gn * 512], EXP
                                )
                                for b2 in range(gn):
                                    d = g0 + b2 - 4 * j
                                    if d >= 0:
                                        sl = pt[:, b2 * 512:(b2 + 1) * 512]
                                        nc.vector.tensor_mul(sl, sl, MASK[:, d, :])
                                pts[h] = pt
                            for b2 in range(gn):
                                i = g0 + b2
                                for h in heads:
                                    nc.tensor.matmul(
                                        oaccs[h],
                                        VA[:, i, h * 65:h * 65 + 65],
                                        pts[h][:, b2 * 512:(b2 + 1) * 512],
                                        start=(i == 0),
                                        stop=(i == nb - 1),
                                    )
                        for h in heads:
                            po = (h % 2) * 64
                            mc = h // 2
                            # extract sum-of-exp row on ACT (lane-count
                            # agnostic), invert on DVE in SBUF, then
                            # DMA-replicate across 64 partitions.
                            rc = nrm.tile([65, 512], F32, tag="rc")
                            nc.scalar.activation(
                                rc[64:65, :], oaccs[h][64:65, :],
                                mybir.ActivationFunctionType.Copy,
                            )
                            nc.vector.reciprocal(rc[64:65, :], rc[64:65, :])
                            rb = nrm.tile([64, 512], F32, tag="rb")
                            row = rc[64:65, :]
                            row_bcast = bass.AP(
                                tensor=row.tensor,
                                offset=row.offset,
                                ap=[row.ap[0], [0, 64]] + row.ap[1:],
                            )
                            nc.gpsimd.dma_start(out=rb[:], in_=row_bcast)
                            ost = nrm.tile([64, 512], BF16, tag="ost")
                            nc.vector.tensor_mul(ost[:], oaccs[h][0:64, :], rb[:])
                            nc.vector.tensor_scalar_add(ost[:], ost[:], BV[:, h:h + 1])
                            nc.sync.dma_start(
                                OT[po:po + 64, mc, j * 512:(j + 1) * 512], ost[:]
                            )

            # ---------------- Stage C: output projection ----------------
            with (
                tc.tile_pool(name="psC", bufs=2, space="PSUM") as psC,
                tc.tile_pool(name="yst", bufs=3) as yst,
            ):
                jobs = [(mo, n) for mo in range(8) for n in range(4)]
                for g0 in range(0, len(jobs), 3):
                    t3 = psC.tile([128, 1536], F32, tag="psc")
                    for s, (mo, n) in enumerate(jobs[g0:g0 + 3]):
                        psl = t3[:, s * 512:(s + 1) * 512]
                        for kc in range(4):
                            nc.tensor.matmul(
                                psl,
                                WOUT[:, kc, mo * 128:(mo + 1) * 128],
                                OT[:, kc, n * 512:(n + 1) * 512],
                                start=(kc == 0),
                                stop=(kc == 3),
                            )
                        ys = yst.tile([128, 512], F32, tag="ys")
                        nc.vector.tensor_copy(ys[:], psl)
                        nc.sync.dma_start(
                            yT[mo * 128:(mo + 1) * 128, n * 512:(n + 1) * 512], ys[:]
                        )

    nc.compile()
    return nc


def _make_masks():
    p = np.arange(128)[:, None, None]
    d = np.arange(4)[None, :, None] * 128
    f = np.arange(512)[None, None, :]
    return (d + p <= f).astype(np.float32).astype(NPBF16)


def kernel(x, w_qkv, b_qkv, w_out, b_out):
    global _CACHE, LAST_RESULT
    x = np.asarray(x, np.float32)
    w_qkv = np.asarray(w_qkv, np.float32)
    b_qkv = np.asarray(b_qkv, np.float32)
    w_out = np.asarray(w_out, np.float32)
    b_out = np.asarray(b_out, np.float32)

    if _CACHE is None:
        _CACHE = _build()
    nc = _CACHE

    masks = _make_masks()
    in_maps = []
    for core in range(8):
        b = core // 2
        g = core % 2
        sl = slice(g * 512, (g + 1) * 512)
        wq = w_qkv[:, 0:1024][:, sl] * SCALE
        wk = w_qkv[:, 1024:2048][:, sl]
        wv = w_qkv[:, 2048:3072][:, sl]
        wqkv_c = np.ascontiguousarray(
            np.concatenate([wq, wk, wv], axis=1).astype(NPBF16)
        )
        bq = b_qkv[0:1024][sl] * SCALE
        bk = b_qkv[1024:2048][sl]
        bqk_c = np.ascontiguousarray(
            np.concatenate([bq, bk]).reshape(CK, 128).T.astype(np.float32)
        )
        bv_c = np.ascontiguousarray(
            b_qkv[2048:3072][sl].reshape(HC, 64).T.astype(np.float32)
        )
        in_maps.append(
            {
                "xT": np.ascontiguousarray(x[b].T.astype(NPBF16)),
                "wqkv": wqkv_c,
                "bqk": bqk_c,
                "bv": bv_c,
                "wout": np.ascontiguousarray(w_out[sl, :].astype(NPBF16)),
                "msk": masks,
            }
        )

    res = run_bass_kernel_spmd(nc, in_maps, core_ids=list(range(8)), trace=TRACE)
    LAST_RESULT = res

    out = np.empty((B, T, C), np.float32)
    for b in range(B):
        acc = res.results[2 * b]["yT"].astype(np.float32) + res.results[
            2 * b + 1
        ]["yT"].astype(np.float32)
        out[b] = acc.T + b_out[None, :]
    return out


# revision 12
# speedup vs baseline: 1.1921x; 1.0864x over previous
"""Causal self-attention (B=4, T=2048, C=1024, H=16) on 8 TRN2 NeuronCores.

Sharding: core = (batch, head-group) on a 4x2 grid. Each core computes the
attention output of 8 heads for one batch element plus its partial out-proj
(y^T = w_out_slice^T @ out_heads^T); the two head-groups of a batch are summed
on the host (the "out_proj all-reduce"), where the final bias is also added.

On-chip dataflow is fully transposed so no transposes are ever needed:
  qk^T  = w_qkv_slice^T @ x^T          (C on partitions)
  v     = x @ w_v_slice                (T on partitions, natural)
  S^T   = k_h @ q_h^T                  (k-positions on partitions)
  P^T   = exp(S^T) * causal_mask       (no max-subtraction: scores ~ N(0,1))
  outT  = [v|1]^T @ P^T                (ones column accumulates sum-of-exp)
  y^T   = w_out_slice^T @ (outT/sumexp)
"""

import sys
import types

if "/opt/trn_rl_repo" not in sys.path:
    sys.path.insert(0, "/opt/trn_rl_repo")

import numpy as np


def _install_ntff_hook_shim():
    """antenv.axon_hooks is missing in this image; provide it so that
    run_bass_kernel_spmd(trace=True) can capture NTFF profiles."""
    if "antenv.axon_hooks" in sys.modules:
        return
    try:
        from trn_agent_boot.trn_boot import _ntff_profile_via_ctypes

        hook = _ntff_profile_via_ctypes("/opt/axon/libaxon_pjrt.so")
    except Exception:
        hook = None
    m = types.ModuleType("antenv.axon_hooks")
    m.get_axon_ntff_profile_hook = lambda: hook
    sys.modules["antenv.axon_hooks"] = m


_install_ntff_hook_shim()

import concourse.bass as bass  # noqa: E402
from concourse import bacc  # noqa: E402
import concourse.mybir as mybir  # noqa: E402
import concourse.tile as tile  # noqa: E402
from concourse.bass_utils import run_bass_kernel_spmd  # noqa: E402

BF16 = mybir.dt.bfloat16
F32 = mybir.dt.float32
NPBF16 = mybir.dt.np(BF16)
EXP = mybir.ActivationFunctionType.Exp

B, T, C = 4, 2048, 1024
H, DH = 16, 64
HC = 8           # heads per core
CK = C // 128    # 8 contraction chunks over C
TB = T // 128    # 16 key blocks / T row blocks
QC = T // 512    # 4 query chunks
SCALE = 1.0 / np.sqrt(DH)

TRACE = False          # set True (e.g. from test.py) to capture an NTFF profile
LAST_RESULT = None     # BassKernelResults of the last run (exec_time_ns etc.)

_CACHE = None


def _build():
    """Build + compile the single-core Bass program (SPMD across 8 cores).

    Projection and attention are interleaved per head-pair so the Tile
    scheduler can fill PE exp-wait gaps with projection matmuls, and a single
    PSUM pool layout (2x2-bank score tiles + 4 out-accumulators) is used
    throughout -- no PSUM pool transitions.
    """
    nc = bacc.Bacc("TRN2", target_bir_lowering=False, debug=False, num_devices=8)

    xT = nc.dram_tensor("xT", [C, T], BF16, kind="ExternalInput")
    wqkv = nc.dram_tensor("wqkv", [C, 3 * 512], BF16, kind="ExternalInput")
    bqk = nc.dram_tensor("bqk", [128, CK], F32, kind="ExternalInput")
    bv = nc.dram_tensor("bv", [64, HC], F32, kind="ExternalInput")
    wout = nc.dram_tensor("wout", [512, C], BF16, kind="ExternalInput")
    msk = nc.dram_tensor("msk", [128, 4, 512], BF16, kind="ExternalInput")
    yT = nc.dram_tensor("yT", [C, T], F32, kind="ExternalOutput")

    with tile.TileContext(nc) as tc:
        with (
            tc.tile_pool(name="persist", bufs=1) as pp,
            tc.tile_pool(name="sc", bufs=2, space="PSUM") as scp,
            tc.tile_pool(name="oa", bufs=4, space="PSUM") as oap,
            tc.tile_pool(name="pt", bufs=3) as ptp,
            tc.tile_pool(name="nrm", bufs=3) as nrm,
            tc.tile_pool(name="yst", bufs=3) as yst,
        ):
            # per-pair q^T/k^T: head h at partition-offset (h%2)*64; chunk 0=q,
            # chunk 1=k. Per-pair out_heads^T likewise packs 2 heads.
            QKT = [pp.tile([128, 2, T], BF16, tag=f"qkt{p}", name=f"qkt{p}")
                   for p in range(4)]
            OT = [pp.tile([128, T], BF16, tag=f"ot{p}", name=f"ot{p}")
                  for p in range(4)]
            # v blocks, per T-block: 8 heads x (64 v-dims + ones column)
            VA = pp.tile([128, TB, HC * 65], BF16, tag="va")
            MASK = pp.tile([128, 4, 512], BF16, tag="mask")
            WOUT = pp.tile([128, 4, C], BF16, tag="wout")
            BQK = pp.tile([128, CK], F32, tag="bqk")
            BV = pp.tile([64, HC], F32, tag="bv")
            XT = pp.tile([128, CK, T], BF16, tag="xt")
            WQ = pp.tile([128, CK, 1536], BF16, tag="wq")

            nc.sync.dma_start(MASK[:], msk[:])
            nc.sync.dma_start(BQK[:], bqk[:])
            nc.sync.dma_start(BV[:], bv[:])
            for kc in range(4):
                nc.sync.dma_start(WOUT[:, kc, :], wout[kc * 128:(kc + 1) * 128, :])
            for kc in range(CK):
                nc.sync.dma_start(XT[:, kc, :], xT[kc * 128:(kc + 1) * 128, :])
                nc.sync.dma_start(WQ[:, kc, :], wqkv[kc * 128:(kc + 1) * 128, :])
            # ones columns of VA; v copies below overwrite the rest
            nc.vector.memset(VA[:], 1.0)

            def emit_v(psl, t):
                for kc in range(CK):
                    nc.tensor.matmul(
                        psl,
                        XT[:, kc, t * 128:(t + 1) * 128],
                        WQ[:, kc, 1024:1536],
                        start=(kc == 0),
                        stop=(kc == CK - 1),
                    )
                src = psl.rearrange("p (h c) -> p h c", c=64)
                dst = VA[:, t, :].rearrange("p (h c) -> p h c", c=65)[:, :, 0:64]
                nc.vector.tensor_copy(dst, src)

            def emit_qk(psl, pair, qk, n):
                m = pair + 4 * qk  # wqkv column chunk (q: 0-3, k: 4-7)
                for kc in range(CK):
                    nc.tensor.matmul(
                        psl,
                        WQ[:, kc, m * 128:(m + 1) * 128],
                        XT[:, kc, n * 512:(n + 1) * 512],
                        start=(kc == 0),
                        stop=(kc == CK - 1),
                    )
                nc.vector.tensor_scalar_add(
                    QKT[pair][:, qk, n * 512:(n + 1) * 512], psl, BQK[:, m:m + 1]
                )

            # ---- v projection (needed by every pair's PV matmuls) ----
            for t2 in range(0, TB, 2):
                t3 = scp.tile([128, 1024], F32, tag="sc", name="vps")
                emit_v(t3[:, 0:512], t2)
                emit_v(t3[:, 512:1024], t2 + 1)

            # ---- per head-pair: qk projection, then attention ----
            for pair in range(4):
                heads = (2 * pair, 2 * pair + 1)
                qkjobs = [(qk, n) for qk in range(2) for n in range(4)]
                for g0 in range(0, 8, 2):
                    t3 = scp.tile([128, 1024], F32, tag="sc", name="qkps")
                    for s in range(2):
                        qk, n = qkjobs[g0 + s]
                        emit_qk(t3[:, s * 512:(s + 1) * 512], pair, qk, n)

                for j in range(QC):
                    nb = 4 * (j + 1)  # causal: key blocks 0..nb-1
                    oaccs = {
                        h: oap.tile([65, 512], F32, tag="oacc", name=f"oacc{h}")
                        for h in heads
                    }
                    for g0 in range(0, nb, 2):
                        gn = min(2, nb - g0)
                        scs = {
                            h: scp.tile([128, 1024], F32, tag="sc", name=f"sc{h}")
                            for h in heads
                        }
                        # interleave the two heads' K=64 matmuls (PE row groups)
                        for b2 in range(gn):
                            i = g0 + b2
                            for h in heads:
                                po = (h % 2) * 64
                                nc.tensor.matmul(
                                    scs[h][:, b2 * 512:(b2 + 1) * 512],
                                    QKT[pair][po:po + 64, 1, i * 128:(i + 1) * 128],
                                    QKT[pair][po:po + 64, 0, j * 512:(j + 1) * 512],
                                    start=True,
                                    stop=True,
                                )
                        pts = {}
                        for h in heads:
                            pt = ptp.tile([128, 1024], BF16, tag="pt")
                            nc.scalar.activation(
                                pt[:, 0:gn * 512], scs[h][:, 0:gn * 512], EXP
                            )
                            for b2 in range(gn):
                                d = g0 + b2 - 4 * j
                                if d >= 0:
                                    sl = pt[:, b2 * 512:(b2 + 1) * 512]
                                    nc.vector.tensor_mul(sl, sl, MASK[:, d, :])
                            pts[h] = pt
                        for b2 in range(gn):
                            i = g0 + b2
                            for h in heads:
                                nc.tensor.matmul(
                                    oaccs[h],
                                    VA[:, i, h * 65:h * 65 + 65],
                                    pts[h][:, b2 * 512:(b2 + 1) * 512],
                                    start=(i == 0),
                                    stop=(i == nb - 1),
                                )
                    for h in heads:
                        po = (h % 2) * 64
                        # 1/sum-of-exp: extract the accumulated ones-row on ACT
                        # (lane-count agnostic), spread 512 values over 64
                        # partitions so DVE's iterative divide runs wide, then
                        # gather + replicate back to (64, 512).
                        rc = nrm.tile([65, 512], F32, tag="rc")
                        nc.scalar.activation(
                            rc[64:65, :], oaccs[h][64:65, :],
                            mybir.ActivationFunctionType.Copy,
                        )
                        rs = nrm.tile([64, 8], F32, tag="rs")
                        nc.sync.dma_start(out=rs[:], in_=rc[64:65, :])
                        nc.vector.reciprocal(rs[:], rs[:])
                        rr = nrm.tile([1, 512], F32, tag="rr")
                        nc.sync.dma_start(out=rr[:], in_=rs[:])
                        rb = nrm.tile([64, 512], F32, tag="rb")
                        row = rr[0:1, :]
                        row_bcast = bass.AP(
                            tensor=row.tensor,
                            offset=row.offset,
                            ap=[row.ap[0], [0, 64]] + row.ap[1:],
                        )
                        nc.gpsimd.dma_start(out=rb[:], in_=row_bcast)
                        ost = nrm.tile([64, 512], BF16, tag="ost")
                        nc.vector.tensor_mul(ost[:], oaccs[h][0:64, :], rb[:])
                        nc.vector.tensor_scalar_add(ost[:], ost[:], BV[:, h:h + 1])
                        nc.sync.dma_start(
                            OT[pair][po:po + 64, j * 512:(j + 1) * 512], ost[:]
                        )

            # ---- output projection ----
            yjobs = [(mo, n) for mo in range(8) for n in range(4)]
            for g0 in range(0, len(yjobs), 2):
                t3 = scp.tile([128, 1024], F32, tag="sc", name="yps")
                for s, (mo, n) in enumerate(yjobs[g0:g0 + 2]):
                    psl = t3[:, s * 512:(s + 1) * 512]
                    for kc in range(4):
                        nc.tensor.matmul(
                            psl,
                            WOUT[:, kc, mo * 128:(mo + 1) * 128],
                            OT[kc][:, n * 512:(n + 1) * 512],
                            start=(kc == 0),
                            stop=(kc == 3),
                        )
                    ys = yst.tile([128, 512], F32, tag="ys")
                    nc.vector.tensor_copy(ys[:], psl)
                    nc.sync.dma_start(
                        yT[mo * 128:(mo + 1) * 128, n * 512:(n + 1) * 512], ys[:]
                    )

    nc.compile()
    return nc


def _make_masks():
    p = np.arange(128)[:, None, None]
    d = np.arange(4)[None, :, None] * 128
    f = np.arange(512)[None, None, :]
    return (d + p <= f).astype(np.float32).astype(NPBF16)


def kernel(x, w_qkv, b_qkv, w_out, b_out):
    global _CACHE, LAST_RESULT
    x = np.asarray(x, np.float32)
    w_qkv = np.asarray(w_qkv, np.float32)
    b_qkv = np.asarray(b_qkv, np.float32)
    w_out = np.asarray(w_out, np.float32)
    b_out = np.asarray(b_out, np.float32)

    if _CACHE is None:
        _CACHE = _build()
    nc = _CACHE

    masks = _make_masks()
    in_maps = []
    for core in range(8):
        b = core // 2
        g = core % 2
        sl = slice(g * 512, (g + 1) * 512)
        wq = w_qkv[:, 0:1024][:, sl] * SCALE
        wk = w_qkv[:, 1024:2048][:, sl]
        wv = w_qkv[:, 2048:3072][:, sl]
        wqkv_c = np.ascontiguousarray(
            np.concatenate([wq, wk, wv], axis=1).astype(NPBF16)
        )
        bq = b_qkv[0:1024][sl] * SCALE
        bk = b_qkv[1024:2048][sl]
        bqk_c = np.ascontiguousarray(
            np.concatenate([bq, bk]).reshape(CK, 128).T.astype(np.float32)
        )
        bv_c = np.ascontiguousarray(
            b_qkv[2048:3072][sl].reshape(HC, 64).T.astype(np.float32)
        )
        in_maps.append(
            {
                "xT": np.ascontiguousarray(x[b].T.astype(NPBF16)),
                "wqkv": wqkv_c,
                "bqk": bqk_c,
                "bv": bv_c,
                "wout": np.ascontiguousarray(w_out[sl, :].astype(NPBF16)),
                "msk": masks,
            }
        )

    res = run_bass_kernel_spmd(nc, in_maps, core_ids=list(range(8)), trace=TRACE)
    LAST_RESULT = res

    out = np.empty((B, T, C), np.float32)
    for b in range(B):
        acc = res.results[2 * b]["yT"].astype(np.float32) + res.results[
            2 * b + 1
        ]["yT"].astype(np.float32)
        out[b] = acc.T + b_out[None, :]
    return out


# revision 13
# speedup vs baseline: 1.2231x; 1.0260x over previous
"""Causal self-attention (B=4, T=2048, C=1024, H=16) on 8 TRN2 NeuronCores.

Sharding: core = (batch, head-group) on a 4x2 grid. Each core computes the
attention output of 8 heads for one batch element plus its partial out-proj
(y^T = w_out_slice^T @ out_heads^T); the two head-groups of a batch are summed
on the host (the "out_proj all-reduce"), where the final bias is also added.

On-chip dataflow is fully transposed so no transposes are ever needed:
  qk^T  = w_qkv_slice^T @ x^T          (C on partitions)
  v     = x @ w_v_slice                (T on partitions, natural)
  S^T   = k_h @ q_h^T                  (k-positions on partitions)
  P^T   = exp(S^T) * causal_mask       (no max-subtraction: scores ~ N(0,1))
  outT  = [v|1]^T @ P^T                (ones column accumulates sum-of-exp)
  y^T   = w_out_slice^T @ (outT/sumexp)
"""

import sys
import types

if "/opt/trn_rl_repo" not in sys.path:
    sys.path.insert(0, "/opt/trn_rl_repo")

import numpy as np


def _install_ntff_hook_shim():
    """antenv.axon_hooks is missing in this image; provide it so that
    run_bass_kernel_spmd(trace=True) can capture NTFF profiles."""
    if "antenv.axon_hooks" in sys.modules:
        return
    try:
        from trn_agent_boot.trn_boot import _ntff_profile_via_ctypes

        hook = _ntff_profile_via_ctypes("/opt/axon/libaxon_pjrt.so")
    except Exception:
        hook = None
    m = types.ModuleType("antenv.axon_hooks")
    m.get_axon_ntff_profile_hook = lambda: hook
    sys.modules["antenv.axon_hooks"] = m


_install_ntff_hook_shim()

import concourse.bass as bass  # noqa: E402
from concourse import bacc  # noqa: E402
import concourse.mybir as mybir  # noqa: E402
import concourse.tile as tile  # noqa: E402
from concourse.bass_utils import run_bass_kernel_spmd  # noqa: E402

BF16 = mybir.dt.bfloat16
F32 = mybir.dt.float32
NPBF16 = mybir.dt.np(BF16)
EXP = mybir.ActivationFunctionType.Exp

B, T, C = 4, 2048, 1024
H, DH = 16, 64
HC = 8           # heads per core
CK = C // 128    # 8 contraction chunks over C
TB = T // 128    # 16 key blocks / T row blocks
QC = T // 512    # 4 query chunks
SCALE = 1.0 / np.sqrt(DH)

TRACE = False          # set True (e.g. from test.py) to capture an NTFF profile
LAST_RESULT = None     # BassKernelResults of the last run (exec_time_ns etc.)

_CACHE = None


def _build():
    """Build + compile the single-core Bass program (SPMD across 8 cores).

    Every matmul uses the full 128x128 PE tile configuration (no mode
    switches, which cost a PE drain each): k^T is zero-padded per head to 128
    contraction rows (even heads live in rows 0-63, odd heads in rows 64-127,
    matching the packed q^T layout so the zero rows mask the other head), and
    the PV stationary is widened to 128 columns (output rows 65-127 are
    don't-care).  Projection and attention are interleaved per head-pair so
    the Tile scheduler can fill exp-wait gaps with projection matmuls.
    """
    nc = bacc.Bacc("TRN2", target_bir_lowering=False, debug=False, num_devices=8)

    xT = nc.dram_tensor("xT", [C, T], BF16, kind="ExternalInput")
    wqkv = nc.dram_tensor("wqkv", [C, 3 * 512], BF16, kind="ExternalInput")
    bqk = nc.dram_tensor("bqk", [128, CK], F32, kind="ExternalInput")
    bv = nc.dram_tensor("bv", [64, HC], F32, kind="ExternalInput")
    wout = nc.dram_tensor("wout", [512, C], BF16, kind="ExternalInput")
    msk = nc.dram_tensor("msk", [128, 4, 512], BF16, kind="ExternalInput")
    yT = nc.dram_tensor("yT", [C, T], F32, kind="ExternalOutput")

    VROW = HC * 65 + 63  # v block row: 8 x (64 v-dims + ones) + stationary pad

    with tile.TileContext(nc) as tc:
        with (
            tc.tile_pool(name="persist", bufs=1) as pp,
            tc.tile_pool(name="sc", bufs=2, space="PSUM") as scp,
            tc.tile_pool(name="oa", bufs=4, space="PSUM") as oap,
            tc.tile_pool(name="pt", bufs=3) as ptp,
            tc.tile_pool(name="nrm", bufs=2) as nrm,
            tc.tile_pool(name="yst", bufs=3) as yst,
        ):
            # q^T packed per pair: head h in partitions (h%2)*64..; k^T padded
            # per head to full 128 contraction rows (other head's rows zero).
            QT = [pp.tile([128, T], BF16, tag=f"qt{p}", name=f"qt{p}")
                  for p in range(4)]
            KP = [pp.tile([128, T], BF16, tag=f"kp{h}", name=f"kp{h}")
                  for h in range(HC)]
            OT = [pp.tile([128, T], BF16, tag=f"ot{p}", name=f"ot{p}")
                  for p in range(4)]
            VA = pp.tile([128, TB, VROW], BF16, tag="va")
            MASK = pp.tile([128, 4, 512], BF16, tag="mask")
            WOUT = pp.tile([128, 4, C], BF16, tag="wout")
            BQK = pp.tile([128, CK], F32, tag="bqk")
            BV = pp.tile([64, HC], F32, tag="bv")
            XT = pp.tile([128, CK, T], BF16, tag="xt")
            WQ = pp.tile([128, CK, 1536], BF16, tag="wq")

            nc.sync.dma_start(MASK[:], msk[:])
            nc.sync.dma_start(BQK[:], bqk[:])
            nc.sync.dma_start(BV[:], bv[:])
            for kc in range(4):
                nc.sync.dma_start(WOUT[:, kc, :], wout[kc * 128:(kc + 1) * 128, :])
            # column-quarter order so the first v/qk projections start early
            for n in range(4):
                for kc in range(CK):
                    nc.sync.dma_start(
                        XT[:, kc, n * 512:(n + 1) * 512],
                        xT[kc * 128:(kc + 1) * 128, n * 512:(n + 1) * 512],
                    )
                    nc.sync.dma_start(
                        WQ[:, kc, n * 384:(n + 1) * 384],
                        wqkv[kc * 128:(kc + 1) * 128, n * 384:(n + 1) * 384],
                    )
            # ones columns of VA; v copies below overwrite the v columns
            nc.vector.memset(VA[:], 1.0)
            # zero halves of the padded k^T tiles (gpsimd: off the DVE)
            for h in range(HC):
                po = (h % 2) * 64
                nc.gpsimd.memset(KP[h][64 - po:128 - po, :], 0.0)

            def emit_v(psl, t):
                for kc in range(CK):
                    nc.tensor.matmul(
                        psl,
                        XT[:, kc, t * 128:(t + 1) * 128],
                        WQ[:, kc, 1024:1536],
                        start=(kc == 0),
                        stop=(kc == CK - 1),
                    )
                src = psl.rearrange("p (h c) -> p h c", c=64)
                dst = VA[:, t, 0:520].rearrange("p (h c) -> p h c", c=65)[:, :, 0:64]
                nc.vector.tensor_copy(dst, src)

            def emit_qk(psl, pair, qk, n):
                m = pair + 4 * qk  # wqkv column chunk (q: 0-3, k: 4-7)
                for kc in range(CK):
                    nc.tensor.matmul(
                        psl,
                        WQ[:, kc, m * 128:(m + 1) * 128],
                        XT[:, kc, n * 512:(n + 1) * 512],
                        start=(kc == 0),
                        stop=(kc == CK - 1),
                    )
                ns = slice(n * 512, (n + 1) * 512)
                if qk == 0:
                    nc.vector.tensor_scalar_add(
                        QT[pair][:, ns], psl, BQK[:, m:m + 1]
                    )
                else:
                    # split per head into the padded k^T tiles (lane-aligned)
                    nc.vector.tensor_scalar_add(
                        KP[2 * pair][0:64, ns], psl[0:64, :], BQK[0:64, m:m + 1]
                    )
                    nc.vector.tensor_scalar_add(
                        KP[2 * pair + 1][64:128, ns], psl[64:128, :],
                        BQK[64:128, m:m + 1],
                    )

            # ---- v projection (needed by every pair's PV matmuls) ----
            for t2 in range(0, TB, 2):
                t3 = scp.tile([128, 1024], F32, tag="sc", name="vps")
                emit_v(t3[:, 0:512], t2)
                emit_v(t3[:, 512:1024], t2 + 1)

            # ---- per head-pair: qk projection, then attention ----
            for pair in range(4):
                heads = (2 * pair, 2 * pair + 1)
                qkjobs = [(qk, n) for qk in range(2) for n in range(4)]
                for g0 in range(0, 8, 2):
                    t3 = scp.tile([128, 1024], F32, tag="sc", name="qkps")
                    for s in range(2):
                        qk, n = qkjobs[g0 + s]
                        emit_qk(t3[:, s * 512:(s + 1) * 512], pair, qk, n)

                for j in range(QC):
                    nb = 4 * (j + 1)  # causal: key blocks 0..nb-1
                    oaccs = {
                        h: oap.tile([128, 512], F32, tag="oacc", name=f"oacc{h}")
                        for h in heads
                    }
                    for g0 in range(0, nb, 2):
                        gn = min(2, nb - g0)
                        scs = {
                            h: scp.tile([128, 1024], F32, tag="sc", name=f"sc{h}")
                            for h in heads
                        }
                        for b2 in range(gn):
                            i = g0 + b2
                            for h in heads:
                                nc.tensor.matmul(
                                    scs[h][:, b2 * 512:(b2 + 1) * 512],
                                    KP[h][:, i * 128:(i + 1) * 128],
                                    QT[pair][:, j * 512:(j + 1) * 512],
                                    start=True,
                                    stop=True,
                                )
                        pts = {}
                        for h in heads:
                            pt = ptp.tile([128, 1024], BF16, tag="pt")
                            nc.scalar.activation(
                                pt[:, 0:gn * 512], scs[h][:, 0:gn * 512], EXP
                            )
                            for b2 in range(gn):
                                d = g0 + b2 - 4 * j
                                if d >= 0:
                                    sl = pt[:, b2 * 512:(b2 + 1) * 512]
                                    nc.vector.tensor_mul(sl, sl, MASK[:, d, :])
                            pts[h] = pt
                        for b2 in range(gn):
                            i = g0 + b2
                            for h in heads:
                                nc.tensor.matmul(
                                    oaccs[h],
                                    VA[:, i, h * 65:h * 65 + 128],
                                    pts[h][:, b2 * 512:(b2 + 1) * 512],
                                    start=(i == 0),
                                    stop=(i == nb - 1),
                                )
                    for h in heads:
                        po = (h % 2) * 64
                        # free the accumulator fast: pull the ones-row (ACT)
                        # and the 64 out rows (DVE) out of PSUM immediately
                        rc = nrm.tile([65, 512], F32, tag="rc")
                        nc.scalar.activation(
                            rc[64:65, :], oaccs[h][64:65, :],
                            mybir.ActivationFunctionType.Copy,
                        )
                        ocp = nrm.tile([64, 512], F32, tag="ocp")
                        nc.vector.tensor_copy(ocp[:], oaccs[h][0:64, :])
                        # 1/sum-of-exp: spread 512 sums over 64 partitions so
                        # DVE's iterative divide runs wide, then gather back
                        # and replicate across partitions.
                        rs = nrm.tile([64, 8], F32, tag="rs")
                        nc.sync.dma_start(out=rs[:], in_=rc[64:65, :])
                        nc.vector.reciprocal(rs[:], rs[:])
                        rr = nrm.tile([1, 512], F32, tag="rr")
                        nc.sync.dma_start(out=rr[:], in_=rs[:])
                        rb = nrm.tile([64, 512], F32, tag="rb")
                        row = rr[0:1, :]
                        row_bcast = bass.AP(
                            tensor=row.tensor,
                            offset=row.offset,
                            ap=[row.ap[0], [0, 64]] + row.ap[1:],
                        )
                        nc.gpsimd.dma_start(out=rb[:], in_=row_bcast)
                        ost = nrm.tile([64, 512], BF16, tag="ost")
                        nc.vector.tensor_mul(ost[:], ocp[:], rb[:])
                        nc.vector.tensor_scalar_add(ost[:], ost[:], BV[:, h:h + 1])
                        nc.sync.dma_start(
                            OT[pair][po:po + 64, j * 512:(j + 1) * 512], ost[:]
                        )

            # ---- output projection (n-major: n-slice 0 is unblocked first) ----
            yjobs = [(mo, n) for n in range(4) for mo in range(8)]
            for g0 in range(0, len(yjobs), 2):
                t3 = scp.tile([128, 1024], F32, tag="sc", name="yps")
                for s, (mo, n) in enumerate(yjobs[g0:g0 + 2]):
                    psl = t3[:, s * 512:(s + 1) * 512]
                    for kc in range(4):
                        nc.tensor.matmul(
                            psl,
                            WOUT[:, kc, mo * 128:(mo + 1) * 128],
                            OT[kc][:, n * 512:(n + 1) * 512],
                            start=(kc == 0),
                            stop=(kc == 3),
                        )
                    ys = yst.tile([128, 512], F32, tag="ys")
                    nc.vector.tensor_copy(ys[:], psl)
                    nc.sync.dma_start(
                        yT[mo * 128:(mo + 1) * 128, n * 512:(n + 1) * 512], ys[:]
                    )

    nc.compile()
    return nc


def _make_masks():
    p = np.arange(128)[:, None, None]
    d = np.arange(4)[None, :, None] * 128
    f = np.arange(512)[None, None, :]
    return (d + p <= f).astype(np.float32).astype(NPBF16)


def kernel(x, w_qkv, b_qkv, w_out, b_out):
    global _CACHE, LAST_RESULT
    x = np.asarray(x, np.float32)
    w_qkv = np.asarray(w_qkv, np.float32)
    b_qkv = np.asarray(b_qkv, np.float32)
    w_out = np.asarray(w_out, np.float32)
    b_out = np.asarray(b_out, np.float32)

    if _CACHE is None:
        _CACHE = _build()
    nc = _CACHE

    masks = _make_masks()
    in_maps = []
    for core in range(8):
        b = core // 2
        g = core % 2
        sl = slice(g * 512, (g + 1) * 512)
        wq = w_qkv[:, 0:1024][:, sl] * SCALE
        wk = w_qkv[:, 1024:2048][:, sl]
        wv = w_qkv[:, 2048:3072][:, sl]
        wqkv_c = np.ascontiguousarray(
            np.concatenate([wq, wk, wv], axis=1).astype(NPBF16)
        )
        bq = b_qkv[0:1024][sl] * SCALE
        bk = b_qkv[1024:2048][sl]
        bqk_c = np.ascontiguousarray(
            np.concatenate([bq, bk]).reshape(CK, 128).T.astype(np.float32)
        )
        bv_c = np.ascontiguousarray(
            b_qkv[2048:3072][sl].reshape(HC, 64).T.astype(np.float32)
        )
        in_maps.append(
            {
                "xT": np.ascontiguousarray(x[b].T.astype(NPBF16)),
                "wqkv": wqkv_c,
                "bqk": bqk_c,
                "bv": bv_c,
                "wout": np.ascontiguousarray(w_out[sl, :].astype(NPBF16)),
                "msk": masks,
            }
        )

    res = run_bass_kernel_spmd(nc, in_maps, core_ids=list(range(8)), trace=TRACE)
    LAST_RESULT = res

    out = np.empty((B, T, C), np.float32)
    for b in range(B):
        acc = res.results[2 * b]["yT"].astype(np.float32) + res.results[
            2 * b + 1
        ]["yT"].astype(np.float32)
        out[b] = acc.T + b_out[None, :]
    return out


# revision 14
# speedup vs baseline: 1.5183x; 1.2414x over previous
"""Causal self-attention (B=4, T=2048, C=1024, H=16) on 8 TRN2 NeuronCores.

Sharding: core = (batch, head-group) on a 4x2 grid. Each core computes the
attention output of 8 heads for one batch element plus its partial out-proj
(y^T = w_out_slice^T @ out_heads^T); the two head-groups of a batch are summed
on the host (the "out_proj all-reduce"), where the final bias is also added.

On-chip dataflow is fully transposed so no transposes are ever needed:
  qk^T  = w_qkv_slice^T @ x^T          (C on partitions)
  v     = x @ w_v_slice                (T on partitions, natural)
  S^T   = k_h @ q_h^T                  (k-positions on partitions)
  P^T   = exp(S^T) * causal_mask       (no max-subtraction: scores ~ N(0,1))
  outT  = [v|1]^T @ P^T                (ones column accumulates sum-of-exp)
  y^T   = w_out_slice^T @ (outT/sumexp)
"""

import sys
import types

if "/opt/trn_rl_repo" not in sys.path:
    sys.path.insert(0, "/opt/trn_rl_repo")

import numpy as np


def _install_ntff_hook_shim():
    """antenv.axon_hooks is missing in this image; provide it so that
    run_bass_kernel_spmd(trace=True) can capture NTFF profiles."""
    if "antenv.axon_hooks" in sys.modules:
        return
    try:
        from trn_agent_boot.trn_boot import _ntff_profile_via_ctypes

        hook = _ntff_profile_via_ctypes("/opt/axon/libaxon_pjrt.so")
    except Exception:
        hook = None
    m = types.ModuleType("antenv.axon_hooks")
    m.get_axon_ntff_profile_hook = lambda: hook
    sys.modules["antenv.axon_hooks"] = m


_install_ntff_hook_shim()

import concourse.bass as bass  # noqa: E402
from concourse import bacc  # noqa: E402
import concourse.mybir as mybir  # noqa: E402
import concourse.tile as tile  # noqa: E402
from concourse.bass_utils import run_bass_kernel_spmd  # noqa: E402

BF16 = mybir.dt.bfloat16
F32 = mybir.dt.float32
NPBF16 = mybir.dt.np(BF16)
EXP = mybir.ActivationFunctionType.Exp

B, T, C = 4, 2048, 1024
H, DH = 16, 64
HC = 8           # heads per core
CK = C // 128    # 8 contraction chunks over C
TB = T // 128    # 16 key blocks / T row blocks
QC = T // 512    # 4 query chunks
SCALE = 1.0 / np.sqrt(DH)

TRACE = False          # set True (e.g. from test.py) to capture an NTFF profile
LAST_RESULT = None     # BassKernelResults of the last run (exec_time_ns etc.)

_CACHE = None


def _build():
    """Build + compile the single-core Bass program (SPMD across 8 cores).

    Every matmul uses the full 128x128 PE tile configuration (no mode
    switches, which cost a PE drain each): k^T is zero-padded per head to 128
    contraction rows (even heads live in rows 0-63, odd heads in rows 64-127,
    matching the packed q^T layout so the zero rows mask the other head), and
    the PV stationary is widened to 128 columns (output rows 65-127 are
    don't-care).  Projection and attention are interleaved per head-pair so
    the Tile scheduler can fill exp-wait gaps with projection matmuls.
    """
    nc = bacc.Bacc("TRN2", target_bir_lowering=False, debug=False, num_devices=8)

    xT = nc.dram_tensor("xT", [C, T], BF16, kind="ExternalInput")
    wqkv = nc.dram_tensor("wqkv", [C, 3 * 512], BF16, kind="ExternalInput")
    bqk = nc.dram_tensor("bqk", [128, CK], F32, kind="ExternalInput")
    bv = nc.dram_tensor("bv", [64, HC], F32, kind="ExternalInput")
    wout = nc.dram_tensor("wout", [512, C], BF16, kind="ExternalInput")
    msk = nc.dram_tensor("msk", [128, 4, 512], BF16, kind="ExternalInput")
    yT = nc.dram_tensor("yT", [C, T], F32, kind="ExternalOutput")

    VROW = HC * 65 + 63  # v block row: 8 x (64 v-dims + ones) + stationary pad

    with tile.TileContext(nc) as tc:
        with (
            tc.tile_pool(name="persist", bufs=1) as pp,
            tc.tile_pool(name="sc", bufs=2, space="PSUM") as scp,
            tc.tile_pool(name="oa", bufs=4, space="PSUM") as oap,
            tc.tile_pool(name="pt", bufs=3) as ptp,
            tc.tile_pool(name="nrm", bufs=2) as nrm,
            tc.tile_pool(name="yst", bufs=3) as yst,
        ):
            # q^T packed per pair: head h in partitions (h%2)*64..; k^T padded
            # per head to full 128 contraction rows (other head's rows zero).
            QT = [pp.tile([128, T], BF16, tag=f"qt{p}", name=f"qt{p}")
                  for p in range(4)]
            KP = [pp.tile([128, T], BF16, tag=f"kp{h}", name=f"kp{h}")
                  for h in range(HC)]
            OT = [pp.tile([128, T], BF16, tag=f"ot{p}", name=f"ot{p}")
                  for p in range(4)]
            VA = pp.tile([128, TB, VROW], BF16, tag="va")
            MASK = pp.tile([128, 4, 512], BF16, tag="mask")
            WOUT = pp.tile([128, 4, C], BF16, tag="wout")
            BQK = pp.tile([128, CK], F32, tag="bqk")
            BV = pp.tile([64, HC], F32, tag="bv")
            XT = pp.tile([128, CK, T], BF16, tag="xt")
            WQ = pp.tile([128, CK, 1536], BF16, tag="wq")

            nc.sync.dma_start(MASK[:], msk[:])
            nc.sync.dma_start(BQK[:], bqk[:])
            nc.sync.dma_start(BV[:], bv[:])
            for kc in range(4):
                nc.sync.dma_start(WOUT[:, kc, :], wout[kc * 128:(kc + 1) * 128, :])
            # v-weight columns + x quarters first so v-projection starts early
            for kc in range(CK):
                nc.sync.dma_start(
                    WQ[:, kc, 1024:1536], wqkv[kc * 128:(kc + 1) * 128, 1024:1536]
                )
            for n in range(4):
                for kc in range(CK):
                    nc.sync.dma_start(
                        XT[:, kc, n * 512:(n + 1) * 512],
                        xT[kc * 128:(kc + 1) * 128, n * 512:(n + 1) * 512],
                    )
            for pair in range(4):
                for kc in range(CK):
                    nc.sync.dma_start(
                        WQ[:, kc, pair * 128:(pair + 1) * 128],
                        wqkv[kc * 128:(kc + 1) * 128, pair * 128:(pair + 1) * 128],
                    )
                    nc.sync.dma_start(
                        WQ[:, kc, 512 + pair * 128:512 + (pair + 1) * 128],
                        wqkv[kc * 128:(kc + 1) * 128,
                             512 + pair * 128:512 + (pair + 1) * 128],
                    )
            # all-ones stationary for the PE-based sum-of-exp broadcast, and
            # two zero-padded moving tiles (row 0 carries 1/sumexp; zero rows
            # make onesT.T @ rr replicate row 0 to all 128 output partitions)
            ONES = pp.tile([128, 128], F32, tag="ones")
            nc.vector.memset(ONES[:], 1.0)
            RRP = [pp.tile([128, 512], F32, tag=f"rrp{i}", name=f"rrp{i}")
                   for i in range(2)]
            nc.vector.memset(RRP[0][:], 0.0)
            nc.vector.memset(RRP[1][:], 0.0)
            # ones columns of VA; v copies below overwrite the v columns
            nc.vector.memset(VA[:], 1.0)
            # zero halves of the padded k^T tiles (gpsimd: off the DVE)
            for h in range(HC):
                po = (h % 2) * 64
                nc.gpsimd.memset(KP[h][64 - po:128 - po, :], 0.0)

            def emit_v(psl, t):
                for kc in range(CK):
                    nc.tensor.matmul(
                        psl,
                        XT[:, kc, t * 128:(t + 1) * 128],
                        WQ[:, kc, 1024:1536],
                        start=(kc == 0),
                        stop=(kc == CK - 1),
                    )
                src = psl.rearrange("p (h c) -> p h c", c=64)
                dst = VA[:, t, 0:520].rearrange("p (h c) -> p h c", c=65)[:, :, 0:64]
                nc.vector.tensor_copy(dst, src)

            def emit_qk(psl, pair, qk, n):
                m = pair + 4 * qk  # wqkv column chunk (q: 0-3, k: 4-7)
                for kc in range(CK):
                    nc.tensor.matmul(
                        psl,
                        WQ[:, kc, m * 128:(m + 1) * 128],
                        XT[:, kc, n * 512:(n + 1) * 512],
                        start=(kc == 0),
                        stop=(kc == CK - 1),
                    )
                ns = slice(n * 512, (n + 1) * 512)
                if qk == 0:
                    nc.vector.tensor_scalar_add(
                        QT[pair][:, ns], psl, BQK[:, m:m + 1]
                    )
                else:
                    # split per head into the padded k^T tiles (lane-aligned)
                    nc.vector.tensor_scalar_add(
                        KP[2 * pair][0:64, ns], psl[0:64, :], BQK[0:64, m:m + 1]
                    )
                    nc.vector.tensor_scalar_add(
                        KP[2 * pair + 1][64:128, ns], psl[64:128, :],
                        BQK[64:128, m:m + 1],
                    )

            # ---- v projection (needed by every pair's PV matmuls) ----
            for t2 in range(0, TB, 2):
                t3 = scp.tile([128, 1024], F32, tag="sc", name="vps")
                emit_v(t3[:, 0:512], t2)
                emit_v(t3[:, 512:1024], t2 + 1)

            # ---- per head-pair: qk projection, then attention ----
            for pair in range(4):
                heads = (2 * pair, 2 * pair + 1)
                qkjobs = [(qk, n) for qk in range(2) for n in range(4)]
                for g0 in range(0, 8, 2):
                    t3 = scp.tile([128, 1024], F32, tag="sc", name="qkps")
                    for s in range(2):
                        qk, n = qkjobs[g0 + s]
                        emit_qk(t3[:, s * 512:(s + 1) * 512], pair, qk, n)

                for j in range(QC):
                    nb = 4 * (j + 1)  # causal: key blocks 0..nb-1
                    oaccs = {
                        h: oap.tile([128, 512], F32, tag="oacc", name=f"oacc{h}")
                        for h in heads
                    }
                    for g0 in range(0, nb, 2):
                        gn = min(2, nb - g0)
                        scs = {
                            h: scp.tile([128, 1024], F32, tag="sc", name=f"sc{h}")
                            for h in heads
                        }
                        for b2 in range(gn):
                            i = g0 + b2
                            for h in heads:
                                nc.tensor.matmul(
                                    scs[h][:, b2 * 512:(b2 + 1) * 512],
                                    KP[h][:, i * 128:(i + 1) * 128],
                                    QT[pair][:, j * 512:(j + 1) * 512],
                                    start=True,
                                    stop=True,
                                )
                        pts = {}
                        for h in heads:
                            pt = ptp.tile([128, 1024], BF16, tag="pt")
                            nc.scalar.activation(
                                pt[:, 0:gn * 512], scs[h][:, 0:gn * 512], EXP
                            )
                            for b2 in range(gn):
                                d = g0 + b2 - 4 * j
                                if d >= 0:
                                    sl = pt[:, b2 * 512:(b2 + 1) * 512]
                                    nc.vector.tensor_mul(sl, sl, MASK[:, d, :])
                            pts[h] = pt
                        for b2 in range(gn):
                            i = g0 + b2
                            for h in heads:
                                nc.tensor.matmul(
                                    oaccs[h],
                                    VA[:, i, h * 65:h * 65 + 128],
                                    pts[h][:, b2 * 512:(b2 + 1) * 512],
                                    start=(i == 0),
                                    stop=(i == nb - 1),
                                )
                    for h in heads:
                        po = (h % 2) * 64
                        # pull the ones-row (ACT) and the 64 out rows (DVE)
                        # out of PSUM immediately
                        rc = nrm.tile([65, 512], F32, tag="rc")
                        nc.scalar.activation(
                            rc[64:65, :], oaccs[h][64:65, :],
                            mybir.ActivationFunctionType.Copy,
                        )
                        ocp = nrm.tile([64, 512], F32, tag="ocp")
                        nc.vector.tensor_copy(ocp[:], oaccs[h][0:64, :])
                        # 1/sum-of-exp: spread 512 sums over 64 partitions so
                        # DVE's iterative divide runs wide, gather into row 0
                        # of a zero-padded tile, then replicate across
                        # partitions with a PE matmul (onesT.T @ rr) reusing
                        # the accumulator bank as the broadcast target.
                        rs = nrm.tile([64, 8], F32, tag="rs")
                        nc.sync.dma_start(out=rs[:], in_=rc[64:65, :])
                        nc.vector.reciprocal(rs[:], rs[:])
                        rrp = RRP[(pair * 8 + j * 2 + h % 2) % 2]
                        nc.sync.dma_start(out=rrp[0:1, :], in_=rs[:])
                        nc.tensor.matmul(
                            oaccs[h][:, :], ONES[:], rrp[:], start=True, stop=True
                        )
                        ost = nrm.tile([64, 512], BF16, tag="ost")
                        nc.vector.tensor_mul(ost[:], ocp[:], oaccs[h][0:64, :])
                        nc.vector.tensor_scalar_add(ost[:], ost[:], BV[:, h:h + 1])
                        nc.sync.dma_start(
                            OT[pair][po:po + 64, j * 512:(j + 1) * 512], ost[:]
                        )

            # ---- output projection (n-major: n-slice 0 is unblocked first) ----
            yjobs = [(mo, n) for n in range(4) for mo in range(8)]
            for g0 in range(0, len(yjobs), 2):
                t3 = scp.tile([128, 1024], F32, tag="sc", name="yps")
                for s, (mo, n) in enumerate(yjobs[g0:g0 + 2]):
                    psl = t3[:, s * 512:(s + 1) * 512]
                    for kc in range(4):
                        nc.tensor.matmul(
                            psl,
                            WOUT[:, kc, mo * 128:(mo + 1) * 128],
                            OT[kc][:, n * 512:(n + 1) * 512],
                            start=(kc == 0),
                            stop=(kc == 3),
                        )
                    ys = yst.tile([128, 512], F32, tag="ys")
                    nc.vector.tensor_copy(ys[:], psl)
                    nc.sync.dma_start(
                        yT[mo * 128:(mo + 1) * 128, n * 512:(n + 1) * 512], ys[:]
                    )

    nc.compile()
    return nc


def _make_masks():
    p = np.arange(128)[:, None, None]
    d = np.arange(4)[None, :, None] * 128
    f = np.arange(512)[None, None, :]
    return (d + p <= f).astype(np.float32).astype(NPBF16)


def kernel(x, w_qkv, b_qkv, w_out, b_out):
    global _CACHE, LAST_RESULT
    x = np.asarray(x, np.float32)
    w_qkv = np.asarray(w_qkv, np.float32)
    b_qkv = np.asarray(b_qkv, np.float32)
    w_out = np.asarray(w_out, np.float32)
    b_out = np.asarray(b_out, np.float32)

    if _CACHE is None:
        _CACHE = _build()
    nc = _CACHE

    masks = _make_masks()
    in_maps = []
    for core in range(8):
        b = core // 2
        g = core % 2
        sl = slice(g * 512, (g + 1) * 512)
        wq = w_qkv[:, 0:1024][:, sl] * SCALE
        wk = w_qkv[:, 1024:2048][:, sl]
        wv = w_qkv[:, 2048:3072][:, sl]
        wqkv_c = np.ascontiguousarray(
            np.concatenate([wq, wk, wv], axis=1).astype(NPBF16)
        )
        bq = b_qkv[0:1024][sl] * SCALE
        bk = b_qkv[1024:2048][sl]
        bqk_c = np.ascontiguousarray(
            np.concatenate([bq, bk]).reshape(CK, 128).T.astype(np.float32)
        )
        bv_c = np.ascontiguousarray(
            b_qkv[2048:3072][sl].reshape(HC, 64).T.astype(np.float32)
        )
        in_maps.append(
            {
                "xT": np.ascontiguousarray(x[b].T.astype(NPBF16)),
                "wqkv": wqkv_c,
                "bqk": bqk_c,
                "bv": bv_c,
                "wout": np.ascontiguousarray(w_out[sl, :].astype(NPBF16)),
                "msk": masks,
            }
        )

    res = run_bass_kernel_spmd(nc, in_maps, core_ids=list(range(8)), trace=TRACE)
    LAST_RESULT = res

    out = np.empty((B, T, C), np.float32)
    for b in range(B):
        acc = res.results[2 * b]["yT"].astype(np.float32) + res.results[
            2 * b + 1
        ]["yT"].astype(np.float32)
        out[b] = acc.T + b_out[None, :]
    return out
